# revision 16
# baseline (speedup 1.0000x reference)
"""Trainium2 Bass kernel for nn_DifferentiableSampler.

Reference computation (per batch b, sample j):
    locs = clip(point[b,j] + offset[b,j], 0, L-1)
    idx0 = floor(locs); idx1 = ceil(locs)
    w1 = locs - idx0; w0 = 1 - w1
    out[b, j, :] = w0 * input[b, :, idx0] + w1 * input[b, :, idx1]

Strategy (pure data parallel over batch, 4 batches per NeuronCore):
  - stream input[b] (C=256, L=8192) f32 into SBUF as two [128, L] tiles
    in the NATIVE channel-major layout (no transpose, no DRAM scratch)
  - load point/offset contiguously, PE-transpose on-chip into the
    16-partition wrap layout (for gather indices) and the 128-partition
    chunk layout (for interpolation weights)
  - idx0 = min(floor(locs), L-2) via a rounding-mode-agnostic floor;
    combined index list [idx0 ; idx0+1] is PE-replicated to all 128
    partitions and converted to i16
  - gpsimd.ap_gather (Pool-engine SBUF gather along the free dim) pulls
    g0|g1 = input[b, half, idx] for all 4096 indices in one shot per
    half: out[c, i] = in[c, idx_i].  This keeps the gather entirely off
    the DMA engines (the bandwidth bottleneck).
  - per 128-sample block: 4 PE transposes land g0/g1 in sample-major
    PSUM [128, 512]; ACT computes wb = g1T * w1 (per-partition scalar),
    DVE computes out = g0T * w0 + wb with one scalar_tensor_tensor
  - DMA result rows (1 KiB per sample) to DRAM

Exact f32 (no bf16 anywhere).  HBM traffic per core: 32 MiB in + 8 MiB
out (vs 88 MiB for the transpose-through-DRAM approach).
"""

import sys

import numpy as np

if "/opt/trn_rl_repo" not in sys.path:
    sys.path.insert(0, "/opt/trn_rl_repo")

from contextlib import ExitStack

import concourse.bacc as bacc
import concourse.tile as tile
from concourse import masks, mybir
from concourse.bass_utils import run_bass_kernel_spmd

AO = mybir.AluOpType
AF = mybir.ActivationFunctionType
F32 = mybir.dt.float32
I16 = mybir.dt.int16
I32 = mybir.dt.int32

N_CORES = 8
B, C, L, N = 32, 256, 8192, 2048
GAMMA = 1.0  # offset scaling factor


def _floor_ops(nc, pool, locs, shape, tag):
    """Rounding-mode-agnostic floor of a non-negative f32 tile.

    Returns i0f = floor(locs) as f32 (exactly integer valued).  Only
    relies on: f32<->i32 casts being exact on integer-valued inputs, and
    r = cast(cast(x)) being in {floor, floor+1} for round-to-nearest or
    truncation.
    """
    ri = pool.tile(shape, I32, tag=f"{tag}_ri")
    nc.vector.tensor_copy(ri[:], locs[:])
    rf = pool.tile(shape, F32, tag=f"{tag}_rf")
    nc.vector.tensor_copy(rf[:], ri[:])
    m = pool.tile(shape, F32, tag=f"{tag}_m")
    nc.vector.tensor_tensor(m[:], rf[:], locs[:], op=AO.is_gt)  # 1.0 if r > x
    i0f = pool.tile(shape, F32, tag=f"{tag}_i0f")
    nc.vector.tensor_tensor(i0f[:], rf[:], m[:], op=AO.subtract)
    return i0f


def _sampler_body(tc, inp, point, offset, out, bpc, c, l, n, reps=1):
    """Emit the sampler program into TileContext tc."""
    nc = tc.nc
    P = 128
    ch = c // P            # channel-half tiles (2)
    n_blk = n // P         # output sample blocks of 128 (16)
    n_slots = n // 16      # wrap-layout free slots (128)

    with ExitStack() as ctx:
        const_pool = ctx.enter_context(tc.tile_pool(name="const", bufs=1))
        inp_pool = ctx.enter_context(tc.tile_pool(name="inp", bufs=2))
        meta_pool = ctx.enter_context(tc.tile_pool(name="meta", bufs=2))
        scr_pool = ctx.enter_context(tc.tile_pool(name="scr", bufs=1))
        mps_pool = ctx.enter_context(tc.tile_pool(name="mps", bufs=1,
                                                  space="PSUM"))
        g_pool = ctx.enter_context(tc.tile_pool(name="g", bufs=1))
        ps_pool = ctx.enter_context(tc.tile_pool(name="ps", bufs=5,
                                                 space="PSUM"))
        wb_pool = ctx.enter_context(tc.tile_pool(name="wb", bufs=4))
        out_pool = ctx.enter_context(tc.tile_pool(name="outp", bufs=2))

        ident = const_pool.tile([P, P], F32)
        masks.make_identity(nc, ident[:])
        # replication matrix R[16, 128]: R[k, p] = 1 if p % 16 == k
        # (R.T @ x broadcasts a 16-partition wrap block to all 8 groups)
        repl = const_pool.tile([16, P], F32)
        nc.vector.memset(repl[:], 0.0)
        for grp in range(8):
            masks.make_identity(nc, repl[0:16, grp * 16:(grp + 1) * 16],
                                nomemset=True)

        def phase_load(b):
            """Issue meta + input DMAs and the idx/weight compute chain for
            batch b.  Returns (in_tiles, idx16, w0j, w1j)."""
            # input halves first (native layout): the big DMAs hit the
            # device with minimal dispatch latency; the tiny meta loads and
            # idx chain still finish long before the first gather needs them
            in_tiles = []
            for h in range(ch):
                t = inp_pool.tile([P, l], F32, tag=f"inp{h}")
                nc.sync.dma_start(t[:], inp[b, h * P:(h + 1) * P, :])
                in_tiles.append(t)

            # natural-128 layout: partition p holds samples 16p..16p+15
            pA = scr_pool.tile([P, n // P], F32, tag="pA")
            oA = scr_pool.tile([P, n // P], F32, tag="oA")
            nc.sync.dma_start(pA[:], point[b].rearrange("(p k) -> p k", p=P))
            nc.sync.dma_start(oA[:], offset[b].rearrange("(p k) -> p k", p=P))
            # natural-16 layout: partition k holds samples 128k..128k+127
            pB = scr_pool.tile([16, n // 16], F32, tag="pB")
            oB = scr_pool.tile([16, n // 16], F32, tag="oB")
            nc.sync.dma_start(pB[:], point[b].rearrange("(k p) -> k p", k=16))
            nc.sync.dma_start(oB[:], offset[b].rearrange("(k p) -> k p", k=16))

            # sum point+offset in SBUF, then transpose once per layout
            sA = scr_pool.tile([P, n // P], F32, tag="sA")
            nc.vector.tensor_tensor(sA[:], pA[:], oA[:], op=AO.add)
            sB = scr_pool.tile([16, n // 16], F32, tag="sB")
            nc.vector.tensor_tensor(sB[:], pB[:], oB[:], op=AO.add)
            # wrap layout [16, n/16]: wrap[q, s] = sample 16s+q = T(natural-128)
            psW = mps_pool.tile([16, n_slots], F32, tag="psW")
            nc.tensor.transpose(psW[:], sA[:], ident[:])
            # chunk layout [128, n/128]: chunk[p, k] = sample 128k+p = T(nat-16)
            psC = mps_pool.tile([P, n_blk], F32, tag="psC")
            nc.tensor.transpose(psC[:], sB[:], ident[0:16, 0:16])

            # indices in wrap layout
            locs_w = scr_pool.tile([16, n_slots], F32, tag="locsw")
            nc.vector.tensor_scalar(locs_w[:], psW[:], 0.0, float(l - 1),
                                    op0=AO.max, op1=AO.min)
            i0f = _floor_ops(nc, scr_pool, locs_w, [16, n_slots], "w")
            nc.vector.tensor_scalar(i0f[:], i0f[:], float(l - 2), None,
                                    op0=AO.min)
            # combined [idx0 ; idx0+1] list, then replicate to 128 partitions
            cv = scr_pool.tile([16, 2 * n_slots], F32, tag="cv")
            nc.vector.tensor_copy(cv[:, 0:n_slots], i0f[:])
            nc.vector.tensor_scalar(cv[:, n_slots:2 * n_slots], i0f[:],
                                    1.0, None, op0=AO.add)
            ps_i = mps_pool.tile([P, 2 * n_slots], F32, tag="psidx")
            nc.tensor.matmul(ps_i[:], repl[:], cv[:])
            idx16 = meta_pool.tile([P, 2 * n_slots], I16, tag="idx16")
            nc.vector.tensor_copy(idx16[:], ps_i[:])

            # weights in chunk layout
            locs_j = scr_pool.tile([P, n_blk], F32, tag="locsj")
            nc.vector.tensor_scalar(locs_j[:], psC[:], 0.0, float(l - 1),
                                    op0=AO.max, op1=AO.min)
            i0fj = _floor_ops(nc, scr_pool, locs_j, [P, n_blk], "j")
            nc.vector.tensor_scalar(i0fj[:], i0fj[:], float(l - 2), None,
                                    op0=AO.min)
            w1j = meta_pool.tile([P, n_blk], F32, tag="w1j")
            nc.vector.tensor_tensor(w1j[:], locs_j[:], i0fj[:],
                                    op=AO.subtract)
            w0j = meta_pool.tile([P, n_blk], F32, tag="w0j")
            nc.vector.tensor_scalar(w0j[:], w1j[:], -1.0, 1.0,
                                    op0=AO.mult, op1=AO.add)
            return in_tiles, idx16, w0j, w1j

        def phase_compute(b, state, last=False):
            """Gather + transpose + interpolate for batch b; returns the
            deferred output stores.

            Each channel-half runs the full chain independently so the
            half-1 gather overlaps half-0's transposes/interp/stores and
            nothing waits on both gathers at once.  Output rows are written
            in two 512 B half-row pieces (still full DMA descriptor rate).
            """
            in_tiles, idx16, w0j, w1j = state
            stores = []
            out_ap = out[b].rearrange("(k p) c -> p k c", p=P)
            for h in range(ch):
                # gather this half (Pool engine, off the DMA device)
                g = g_pool.tile([P, 2 * n], F32, tag=f"g{h}")
                nc.gpsimd.ap_gather(
                    g[:], in_tiles[h][:], idx16[:],
                    channels=P, num_elems=l, d=1, num_idxs=2 * n)

                out_big = out_pool.tile([P, n_blk, P], F32, tag=f"outb{h}")
                for k in range(n_blk):
                    ps = ps_pool.tile([P, 2 * P], F32, tag="ps")
                    # sample-major: A = g_idx0^T, B = g_idx1^T
                    nc.tensor.transpose(
                        ps[:, 0:P], g[:, k * P:(k + 1) * P], ident[:])
                    nc.tensor.transpose(
                        ps[:, P:2 * P], g[:, n + k * P:n + (k + 1) * P],
                        ident[:])
                    wb = wb_pool.tile([P, P], F32, tag="wb")
                    nc.scalar.activation(wb[:], ps[:, P:2 * P], AF.Copy,
                                         scale=w1j[:, k:k + 1])
                    nc.vector.scalar_tensor_tensor(
                        out_big[:, k, :], ps[:, 0:P], w0j[:, k:k + 1],
                        wb[:], op0=AO.mult, op1=AO.add)
                # one store per half: keeps total DMAs at 8/batch so the
                # 8 rotating HW DMA sems align role-to-role across batches
                # (a load never waits on a late store's sem).  The store is
                # not issued here: compute returns it so the main loop can
                # defer it one iteration, keeping stores out of the DMA
                # stream ahead of the final batch's input loads.
                if last:
                    qb = 4  # blocks per epilogue store chunk
                    for q in range(n_blk // qb):
                        stores.append((
                            out_ap[:, q * qb:(q + 1) * qb, h * P:(h + 1) * P],
                            out_big[:, q * qb:(q + 1) * qb, :]))
                else:
                    stores.append((out_ap[:, :, h * P:(h + 1) * P], out_big))
            return stores

        def issue_stores(stores):
            # SP, after the next batch's input loads in SP program order:
            # deferred a full iteration, so the interp they wait on is long
            # done (no SEQ stall), and the loads reach the DMA device first
            for dst, t in stores:
                nc.sync.dma_start(dst, t if isinstance(t, type(dst)) else t[:])

        # software pipeline: issue batch b+1's loads/idx chain before batch
        # b's heavy compute so SP streams DMAs and Pool never starves
        for rep in range(reps):
            state = phase_load(0)
            pending = None
            for b in range(bpc):
                next_state = phase_load(b + 1) if b + 1 < bpc else None
                if pending is not None:
                    issue_stores(pending)
                pending = phase_compute(b, state, last=(b == bpc - 1))
                state = next_state
            issue_stores(pending)


def build_nc(bpc, c, l, n, reps=1):
    nc = bacc.Bacc("TRN2", target_bir_lowering=False, debug=False,
                   num_devices=N_CORES)
    inp = nc.dram_tensor("input", [bpc, c, l], F32, kind="ExternalInput").ap()
    point = nc.dram_tensor("point", [bpc, n], F32, kind="ExternalInput").ap()
    offset = nc.dram_tensor("offset", [bpc, n], F32, kind="ExternalInput").ap()
    out = nc.dram_tensor("out", [bpc, n, c], F32, kind="ExternalOutput").ap()
    with tile.TileContext(nc) as tc:
        _sampler_body(tc, inp, point, offset, out, bpc, c, l, n, reps)
    nc.compile()
    return nc


_NC_CACHE = {}


def _get_nc(bpc=B // N_CORES, c=C, l=L, n=N, reps=1):
    key = (bpc, c, l, n, reps)
    if key not in _NC_CACHE:
        _NC_CACHE[key] = build_nc(*key)
    return _NC_CACHE[key]


def run_sharded(input, point, offset, trace=False, **kwargs):
    """Run the SPMD kernel on the full inputs; returns (output, results)."""
    input = np.ascontiguousarray(input, dtype=np.float32)
    point = np.ascontiguousarray(point, dtype=np.float32).reshape(B, N)
    offset = np.ascontiguousarray(offset, dtype=np.float32).reshape(B, N)
    bpc = B // N_CORES
    nc = _get_nc()
    in_maps = [
        {
            "input": input[i * bpc:(i + 1) * bpc],
            "point": point[i * bpc:(i + 1) * bpc],
            "offset": offset[i * bpc:(i + 1) * bpc],
        }
        for i in range(N_CORES)
    ]
    res = run_bass_kernel_spmd(nc, in_maps, core_ids=list(range(N_CORES)),
                               trace=trace, **kwargs)
    outs = [res.results[i]["out"] for i in range(N_CORES)]
    return np.concatenate(outs, axis=0), res


def kernel(input, point, offset):
    out, _ = run_sharded(input, point, offset, trace=False)
    return out


# revision 17
# speedup vs baseline: 1.2959x; 1.2959x over previous
"""Trainium2 Bass kernel for nn_DifferentiableSampler.

Reference computation (per batch b, sample j):
    locs = clip(point[b,j] + offset[b,j], 0, L-1)
    idx0 = floor(locs); idx1 = ceil(locs)
    w1 = locs - idx0; w0 = 1 - w1
    out[b, j, :] = w0 * input[b, :, idx0] + w1 * input[b, :, idx1]

Strategy (pure data parallel over batch, 4 batches per NeuronCore):
  - stream input[b] (C=256, L=8192) f32 into SBUF as two [128, L] tiles
    in the NATIVE channel-major layout (no transpose, no DRAM scratch)
  - load point/offset contiguously, PE-transpose on-chip into the
    16-partition wrap layout (for gather indices) and the 128-partition
    chunk layout (for interpolation weights)
  - idx0 = min(floor(locs), L-2) via a rounding-mode-agnostic floor;
    combined index list [idx0 ; idx0+1] is PE-replicated to all 128
    partitions and converted to i16
  - gpsimd.ap_gather (Pool-engine SBUF gather along the free dim) pulls
    g0|g1 = input[b, half, idx] for all 4096 indices in one shot per
    half: out[c, i] = in[c, idx_i].  This keeps the gather entirely off
    the DMA engines (the bandwidth bottleneck).
  - per 128-sample block: 4 PE transposes land g0/g1 in sample-major
    PSUM [128, 512]; ACT computes wb = g1T * w1 (per-partition scalar),
    DVE computes out = g0T * w0 + wb with one scalar_tensor_tensor
  - DMA result rows (1 KiB per sample) to DRAM

Exact f32 (no bf16 anywhere).  HBM traffic per core: 32 MiB in + 8 MiB
out (vs 88 MiB for the transpose-through-DRAM approach).
"""

import sys

import numpy as np

if "/opt/trn_rl_repo" not in sys.path:
    sys.path.insert(0, "/opt/trn_rl_repo")

from contextlib import ExitStack

import concourse.bacc as bacc
import concourse.tile as tile
from concourse import masks, mybir
from concourse.bass_utils import run_bass_kernel_spmd

AO = mybir.AluOpType
AF = mybir.ActivationFunctionType
F32 = mybir.dt.float32
I16 = mybir.dt.int16
I32 = mybir.dt.int32

N_CORES = 8
B, C, L, N = 32, 256, 8192, 2048
GAMMA = 1.0  # offset scaling factor


def _floor_ops(nc, pool, locs, shape, tag):
    """Rounding-mode-agnostic floor of a non-negative f32 tile.

    Returns i0f = floor(locs) as f32 (exactly integer valued).  Only
    relies on: f32<->i32 casts being exact on integer-valued inputs, and
    r = cast(cast(x)) being in {floor, floor+1} for round-to-nearest or
    truncation.
    """
    ri = pool.tile(shape, I32, tag=f"{tag}_ri")
    nc.vector.tensor_copy(ri[:], locs[:])
    rf = pool.tile(shape, F32, tag=f"{tag}_rf")
    nc.vector.tensor_copy(rf[:], ri[:])
    m = pool.tile(shape, F32, tag=f"{tag}_m")
    nc.vector.tensor_tensor(m[:], rf[:], locs[:], op=AO.is_gt)  # 1.0 if r > x
    i0f = pool.tile(shape, F32, tag=f"{tag}_i0f")
    nc.vector.tensor_tensor(i0f[:], rf[:], m[:], op=AO.subtract)
    return i0f


def _sampler_body(tc, inp, point, offset, out, bpc, c, l, n, reps=1):
    """Emit the sampler program into TileContext tc."""
    nc = tc.nc
    P = 128
    ch = c // P            # channel-half tiles (2)
    n_blk = n // P         # output sample blocks of 128 (16)
    n_slots = n // 16      # wrap-layout free slots (128)

    with ExitStack() as ctx:
        const_pool = ctx.enter_context(tc.tile_pool(name="const", bufs=1))
        inp_pool = ctx.enter_context(tc.tile_pool(name="inp", bufs=2))
        meta_pool = ctx.enter_context(tc.tile_pool(name="meta", bufs=2))
        scr_pool = ctx.enter_context(tc.tile_pool(name="scr", bufs=1))
        mps_pool = ctx.enter_context(tc.tile_pool(name="mps", bufs=1,
                                                  space="PSUM"))
        g_pool = ctx.enter_context(tc.tile_pool(name="g", bufs=1))
        ps_pool = ctx.enter_context(tc.tile_pool(name="ps", bufs=5,
                                                 space="PSUM"))
        wb_pool = ctx.enter_context(tc.tile_pool(name="wb", bufs=4))
        out_pool = ctx.enter_context(tc.tile_pool(name="outp", bufs=2))

        ident = const_pool.tile([P, P], F32)
        masks.make_identity(nc, ident[:])
        # replication matrix R[16, 128]: R[k, p] = 1 if p % 16 == k
        # (R.T @ x broadcasts a 16-partition wrap block to all 8 groups)
        repl = const_pool.tile([16, P], F32)
        nc.vector.memset(repl[:], 0.0)
        for grp in range(8):
            masks.make_identity(nc, repl[0:16, grp * 16:(grp + 1) * 16],
                                nomemset=True)

        def phase_load(b):
            """Issue meta + input DMAs and the idx/weight compute chain for
            batch b.  Returns (in_tiles, idx16, w0j, w1j)."""
            # meta loads first: they never WAR-block, so they don't hold
            # the SP SEQ, and the idx chain can start while inputs stream
            # natural-128 layout: partition p holds samples 16p..16p+15
            pA = scr_pool.tile([P, n // P], F32, tag="pA")
            oA = scr_pool.tile([P, n // P], F32, tag="oA")
            nc.sync.dma_start(pA[:], point[b].rearrange("(p k) -> p k", p=P))
            nc.sync.dma_start(oA[:], offset[b].rearrange("(p k) -> p k", p=P))
            # natural-16 layout: partition k holds samples 128k..128k+127
            pB = scr_pool.tile([16, n // 16], F32, tag="pB")
            oB = scr_pool.tile([16, n // 16], F32, tag="oB")
            nc.sync.dma_start(pB[:], point[b].rearrange("(k p) -> k p", k=16))
            nc.sync.dma_start(oB[:], offset[b].rearrange("(k p) -> k p", k=16))

            # input halves (native layout)
            in_tiles = []
            for h in range(ch):
                t = inp_pool.tile([P, l], F32, tag=f"inp{h}")
                nc.sync.dma_start(t[:], inp[b, h * P:(h + 1) * P, :])
                in_tiles.append(t)

            # sum point+offset in SBUF, then transpose once per layout
            sA = scr_pool.tile([P, n // P], F32, tag="sA")
            nc.vector.tensor_tensor(sA[:], pA[:], oA[:], op=AO.add)
            sB = scr_pool.tile([16, n // 16], F32, tag="sB")
            nc.vector.tensor_tensor(sB[:], pB[:], oB[:], op=AO.add)
            # wrap layout [16, n/16]: wrap[q, s] = sample 16s+q = T(natural-128)
            psW = mps_pool.tile([16, n_slots], F32, tag="psW")
            nc.tensor.transpose(psW[:], sA[:], ident[:])
            # chunk layout [128, n/128]: chunk[p, k] = sample 128k+p = T(nat-16)
            psC = mps_pool.tile([P, n_blk], F32, tag="psC")
            nc.tensor.transpose(psC[:], sB[:], ident[0:16, 0:16])

            # indices in wrap layout
            locs_w = scr_pool.tile([16, n_slots], F32, tag="locsw")
            nc.vector.tensor_scalar(locs_w[:], psW[:], 0.0, float(l - 1),
                                    op0=AO.max, op1=AO.min)
            i0f = _floor_ops(nc, scr_pool, locs_w, [16, n_slots], "w")
            nc.vector.tensor_scalar(i0f[:], i0f[:], float(l - 2), None,
                                    op0=AO.min)
            # combined [idx0 ; idx0+1] list, then replicate to 128 partitions
            cv = scr_pool.tile([16, 2 * n_slots], F32, tag="cv")
            nc.vector.tensor_copy(cv[:, 0:n_slots], i0f[:])
            nc.vector.tensor_scalar(cv[:, n_slots:2 * n_slots], i0f[:],
                                    1.0, None, op0=AO.add)
            ps_i = mps_pool.tile([P, 2 * n_slots], F32, tag="psidx")
            nc.tensor.matmul(ps_i[:], repl[:], cv[:])
            idx16 = meta_pool.tile([P, 2 * n_slots], I16, tag="idx16")
            nc.vector.tensor_copy(idx16[:], ps_i[:])

            # weights in chunk layout
            locs_j = scr_pool.tile([P, n_blk], F32, tag="locsj")
            nc.vector.tensor_scalar(locs_j[:], psC[:], 0.0, float(l - 1),
                                    op0=AO.max, op1=AO.min)
            i0fj = _floor_ops(nc, scr_pool, locs_j, [P, n_blk], "j")
            nc.vector.tensor_scalar(i0fj[:], i0fj[:], float(l - 2), None,
                                    op0=AO.min)
            w1j = meta_pool.tile([P, n_blk], F32, tag="w1j")
            nc.vector.tensor_tensor(w1j[:], locs_j[:], i0fj[:],
                                    op=AO.subtract)
            w0j = meta_pool.tile([P, n_blk], F32, tag="w0j")
            nc.vector.tensor_scalar(w0j[:], w1j[:], -1.0, 1.0,
                                    op0=AO.mult, op1=AO.add)
            return in_tiles, idx16, w0j, w1j

        def phase_compute(b, state, last=False):
            """Gather + transpose + interpolate for batch b; returns the
            deferred output stores.

            Each channel-half runs the full chain independently so the
            half-1 gather overlaps half-0's transposes/interp/stores and
            nothing waits on both gathers at once.  Output rows are written
            in two 512 B half-row pieces (still full DMA descriptor rate).
            """
            in_tiles, idx16, w0j, w1j = state
            stores = []
            out_ap = out[b].rearrange("(k p) c -> p k c", p=P)
            for h in range(ch):
                # gather this half (Pool engine, off the DMA device)
                g = g_pool.tile([P, 2 * n], F32, tag=f"g{h}")
                nc.gpsimd.ap_gather(
                    g[:], in_tiles[h][:], idx16[:],
                    channels=P, num_elems=l, d=1, num_idxs=2 * n)

                out_big = out_pool.tile([P, n_blk, P], F32, tag=f"outb{h}")
                for k in range(n_blk):
                    ps = ps_pool.tile([P, 2 * P], F32, tag="ps")
                    # sample-major: A = g_idx0^T, B = g_idx1^T
                    nc.tensor.transpose(
                        ps[:, 0:P], g[:, k * P:(k + 1) * P], ident[:])
                    nc.tensor.transpose(
                        ps[:, P:2 * P], g[:, n + k * P:n + (k + 1) * P],
                        ident[:])
                    wb = wb_pool.tile([P, P], F32, tag="wb")
                    nc.scalar.activation(wb[:], ps[:, P:2 * P], AF.Copy,
                                         scale=w1j[:, k:k + 1])
                    nc.vector.scalar_tensor_tensor(
                        out_big[:, k, :], ps[:, 0:P], w0j[:, k:k + 1],
                        wb[:], op0=AO.mult, op1=AO.add)
                # one store per half: keeps total DMAs at 8/batch so the
                # 8 rotating HW DMA sems align role-to-role across batches
                # (a load never waits on a late store's sem).  The store is
                # not issued here: compute returns it so the main loop can
                # defer it one iteration, keeping stores out of the DMA
                # stream ahead of the final batch's input loads.
                if last:
                    qb = 4  # blocks per epilogue store chunk
                    for q in range(n_blk // qb):
                        stores.append((
                            out_ap[:, q * qb:(q + 1) * qb, h * P:(h + 1) * P],
                            out_big[:, q * qb:(q + 1) * qb, :]))
                else:
                    stores.append((out_ap[:, :, h * P:(h + 1) * P], out_big))
            return stores

        def issue_stores(stores):
            # SP, after the next batch's input loads in SP program order:
            # deferred a full iteration, so the interp they wait on is long
            # done (no SEQ stall), and the loads reach the DMA device first
            for dst, t in stores:
                nc.sync.dma_start(dst, t if isinstance(t, type(dst)) else t[:])

        # software pipeline: issue batch b+1's loads/idx chain before batch
        # b's heavy compute so SP streams DMAs and Pool never starves
        for rep in range(reps):
            state = phase_load(0)
            pending = None
            for b in range(bpc):
                next_state = phase_load(b + 1) if b + 1 < bpc else None
                if pending is not None:
                    issue_stores(pending)
                pending = phase_compute(b, state, last=(b == bpc - 1))
                state = next_state
            issue_stores(pending)


def build_nc(bpc, c, l, n, reps=1):
    nc = bacc.Bacc("TRN2", target_bir_lowering=False, debug=False,
                   num_devices=N_CORES)
    inp = nc.dram_tensor("input", [bpc, c, l], F32, kind="ExternalInput").ap()
    point = nc.dram_tensor("point", [bpc, n], F32, kind="ExternalInput").ap()
    offset = nc.dram_tensor("offset", [bpc, n], F32, kind="ExternalInput").ap()
    out = nc.dram_tensor("out", [bpc, n, c], F32, kind="ExternalOutput").ap()
    with tile.TileContext(nc) as tc:
        _sampler_body(tc, inp, point, offset, out, bpc, c, l, n, reps)
    nc.compile()
    return nc


_NC_CACHE = {}


def _get_nc(bpc=B // N_CORES, c=C, l=L, n=N, reps=1):
    key = (bpc, c, l, n, reps)
    if key not in _NC_CACHE:
        _NC_CACHE[key] = build_nc(*key)
    return _NC_CACHE[key]


def run_sharded(input, point, offset, trace=False, **kwargs):
    """Run the SPMD kernel on the full inputs; returns (output, results)."""
    input = np.ascontiguousarray(input, dtype=np.float32)
    point = np.ascontiguousarray(point, dtype=np.float32).reshape(B, N)
    offset = np.ascontiguousarray(offset, dtype=np.float32).reshape(B, N)
    bpc = B // N_CORES
    nc = _get_nc()
    in_maps = [
        {
            "input": input[i * bpc:(i + 1) * bpc],
            "point": point[i * bpc:(i + 1) * bpc],
            "offset": offset[i * bpc:(i + 1) * bpc],
        }
        for i in range(N_CORES)
    ]
    res = run_bass_kernel_spmd(nc, in_maps, core_ids=list(range(N_CORES)),
                               trace=trace, **kwargs)
    outs = [res.results[i]["out"] for i in range(N_CORES)]
    return np.concatenate(outs, axis=0), res


def kernel(input, point, offset):
    out, _ = run_sharded(input, point, offset, trace=False)
    return out
